# revision 1
# baseline (speedup 1.0000x reference)
"""Trainium2 Bass kernel for nn_DGN (3x NNConv GNN + all-pairs L1 CBT).

Strategy (8 NeuronCores, SPMD):
  - Edges sorted by (dst, src), sharded so core c owns destination nodes
    [256c, 256(c+1)) as two 128-node chunks; per-(core,chunk) edge lists are
    padded to a uniform tile count so a single SPMD program serves all cores.
  - Per 128-edge tile: PE computes the edge-MLP z = ea' @ W' (bias folded via
    a ones-row), DVE/ACT fuse relu+multiply-by-gathered-source-features, and
    PE scatter-matmuls (one-hot lhsT with 1/deg folded in) accumulate the
    per-node mean *and* set up the i-contraction in a wide [n, out*in] PSUM
    accumulator, reduced at chunk end.  Root-weight term is injected as one
    extra matmul into its own PSUM tile.
  - Source-feature gather runs as a SWDGE dma_gather from an HBM table that
    is rebuilt each layer via an AllGather collective.
  - CBT: per 4-row batch of local output rows, one DVE tensor_scalar
    (subtract, max 0) against a 4x-replicated transposed-h tile produces
    relu(h[j,k]-h[i,k]) for all j,k; a selection matmul on PE sums over k
    into a [128, 2048] PSUM block accumulated over 32 batches, using the
    exact identity sum|d| = 2*sum relu(d) - (R_j - R_i) (the -R_j/2 term is
    one extra fp32 matmul in the same accumulation; +R_i rides the drain).

Status (2026-08-08): passes on HW (rel err ~3.6e-3 vs the 2e-2 gate).
Cost-model (CoreSim) end-to-end: 287 us/core, down from the 346 us
baseline via:
  - z=relu(ea@W) prefetch for the next layer's first chunk issued before
    each AllGather, filling the ~18 us collective windows (PE+ACT+DVE
    would otherwise idle; z depends only on edge_attr, not on h).
  - CBT in bf16: the hTrep diff tensor_scalar hits the DVE 4x perf mode
    (533 ns vs 1067 ns per [128,2048] batch); sel/seln matmuls stream
    bf16 (1 cyc/col vs 4 for f32).
  - Final AllGather ships the already-transposed h3 (each core owns
    hT3), replacing the serial post-collective h3f-DMA + 16 transposes +
    chained replication with one strided DMA + 3 parallel quadrant
    copies (~9 us off the CBT-prep tail).
  - Const loads split: L1-critical tensors first on SP, the rest on the
    idle Pool queue; S streams in 4 chunks so L1 starts on chunk 0.
  - htab replication DMAs spread across SP/ACT/Pool queues; both chunks'
    dma_gathers issued at layer start.
Remaining known structure (sim): L2/L3 bodies are DVE/ACT-bound
(~58 us/layer; relu+mult elementwise floor), CBT is PE-bound (~57 us;
k-reduction streams 16.8M diffs at 128/cycle), 3 collectives at 15 us
fixed cost each with the first two mostly hidden by z-prefetch.
"""

import math
import numpy as np
import ml_dtypes

import concourse.bass as bass
import concourse.bacc as bacc
import concourse.tile as tile
import concourse.mybir as mybir
from concourse.bass_utils import run_bass_kernel_spmd

BF16 = mybir.dt.bfloat16
F32 = mybir.dt.float32
I16 = mybir.dt.int16

N = 2048
E = 65536
NV = 6
C = 32
NCORES = 8
NPC = N // NCORES      # nodes per core = 256
CHUNK = 128            # node chunk (PSUM partition dim)
Op = mybir.AluOpType

_PROGRAM_CACHE = {}


# --------------------------------------------------------------------------
# host-side prep
# --------------------------------------------------------------------------

def _bf16(a):
    return np.asarray(a, dtype=np.float32).astype(ml_dtypes.bfloat16)


def _permute_w(Wf, b):
    """[NV, in*out] + [in*out] -> [7, in*out] with columns re-ordered from
    (i-major) i*out+o to (o-major) o*in+i, bias folded as last row."""
    in_c = Wf.shape[1] // C
    Wb = np.concatenate([Wf, b[None, :]], axis=0)  # [7, in*out]
    cols = Wb.reshape(NV + 1, in_c, C)             # [7, i, o]
    out = np.transpose(cols, (0, 2, 1)).reshape(NV + 1, in_c * C)  # (o, i)
    return out


def _prep(x, edge_attr, edge_index, W1, b1, root1, bias1, W2, b2, root2,
          bias2, W3, b3, root3, bias3):
    src = np.asarray(edge_index[0], dtype=np.int64)
    dst = np.asarray(edge_index[1], dtype=np.int64)
    x = np.asarray(x, dtype=np.float32)
    ea = np.asarray(edge_attr, dtype=np.float32)

    deg = np.bincount(dst, minlength=N).astype(np.float64)
    inv_deg = (1.0 / np.maximum(deg, 1.0)).astype(np.float32)

    order = np.lexsort((src, dst))
    s_srt, d_srt = src[order], dst[order]

    # per (core, chunk) edge index lists (into the original edge arrays)
    groups = []
    for g in range(N // CHUNK):          # 16 chunks
        sel = order[(d_srt >= g * CHUNK) & (d_srt < (g + 1) * CHUNK)]
        groups.append(sel)
    TC = max(1, max((len(g) + 127) // 128 for g in groups))
    Tt = 2 * TC
    Ep = Tt * 128

    per_core = []
    for c in range(NCORES):
        eaT7 = np.zeros((7, Ep), np.float32)
        xsrc = np.zeros((128, Tt), np.float32)
        S = np.zeros((128, Tt * 128), np.float32)
        idx16 = np.zeros((128, Tt * 8), np.int16)
        for ch in range(2):
            g = groups[2 * c + ch]
            n = len(g)
            base = ch * TC * 128
            # edge-major arrays, padded region stays zero
            eaT7[:NV, base:base + n] = ea[g].T
            eaT7[NV, base:base + n] = 1.0
            gs = src[g]
            gd = dst[g]
            for t in range((n + 127) // 128):
                lo, hi = t * 128, min(n, (t + 1) * 128)
                tt = ch * TC + t
                rows = np.arange(lo, hi)
                p = rows - lo
                xsrc[p, tt] = x[gs[rows], 0]
                nl = (gd[rows] - (2 * c + ch) * CHUNK).astype(np.int64)
                S[p, tt * 128 + nl] = 1.0
            # gather indices, wrapped: idx j -> [j%16, j//16], replicated x8
            ids = np.zeros(TC * 128, np.int16)
            ids[:n] = gs.astype(np.int16)
            wrapped = ids.reshape(TC * 8, 16).T          # [16, TC*8]
            idx16[:, ch * TC * 8:(ch + 1) * TC * 8] = np.tile(wrapped, (8, 1))
        xT2 = np.zeros((2, NPC), np.float32)
        xT2[0] = x[c * NPC:(c + 1) * NPC, 0]
        xT2[1] = 1.0
        invd = inv_deg[c * NPC:(c + 1) * NPC].reshape(2, 128).T.copy()
        per_core.append(dict(
            eaT7=_bf16(eaT7), xsrc=xsrc.astype(np.float32), S=_bf16(S),
            idx=idx16, xT2=xT2.astype(np.float32),
            invd=invd.astype(np.float32),
        ))

    selbig = np.zeros((128, 252), np.float32)
    for p in range(128):
        selbig[p, 124 + p // 32] = 1.0
    shared = dict(
        W1p=_bf16(np.concatenate([W1, b1[None, :]], 0)),
        W2p=_bf16(_permute_w(W2, b2)),
        W3p=_bf16(_permute_w(W3, b3)),
        r1=np.stack([root1[0], bias1], 0).astype(np.float32),
        r2=np.concatenate([root2, bias2[None, :]], 0).astype(np.float32),
        r3=np.concatenate([root3, bias3[None, :]], 0).astype(np.float32),
        selbig=_bf16(selbig),
        selneg=_bf16(np.where(np.arange(128)[:, None] < 32, -0.5, 0.0)
                     * np.ones((128, 128))),
        ident=_bf16(np.eye(128, dtype=np.float32)),
        ident32=np.eye(128, dtype=np.float32),
    )
    in_maps = []
    for c in range(NCORES):
        m = dict(per_core[c])
        m.update(shared)
        in_maps.append(m)
    return TC, in_maps


# --------------------------------------------------------------------------
# device program
# --------------------------------------------------------------------------

def build_program(TC):
    Tt = 2 * TC
    Ep = Tt * 128
    nc = bacc.Bacc("TRN2", target_bir_lowering=False, debug=False,
                   num_devices=NCORES)

    d_eaT7 = nc.dram_tensor("eaT7", [7, Ep], BF16, kind="ExternalInput")
    d_xsrc = nc.dram_tensor("xsrc", [128, Tt], F32, kind="ExternalInput")
    d_S = nc.dram_tensor("S", [128, Tt * 128], BF16, kind="ExternalInput")
    d_idx = nc.dram_tensor("idx", [128, Tt * 8], I16, kind="ExternalInput")
    d_xT2 = nc.dram_tensor("xT2", [2, NPC], F32, kind="ExternalInput")
    d_invd = nc.dram_tensor("invd", [128, 2], F32, kind="ExternalInput")
    d_W1 = nc.dram_tensor("W1p", [7, C], BF16, kind="ExternalInput")
    d_W2 = nc.dram_tensor("W2p", [7, C * C], BF16, kind="ExternalInput")
    d_W3 = nc.dram_tensor("W3p", [7, C * C], BF16, kind="ExternalInput")
    d_r1 = nc.dram_tensor("r1", [2, C], F32, kind="ExternalInput")
    d_r2 = nc.dram_tensor("r2", [C + 1, C], F32, kind="ExternalInput")
    d_r3 = nc.dram_tensor("r3", [C + 1, C], F32, kind="ExternalInput")
    d_sel = nc.dram_tensor("selbig", [128, 252], BF16, kind="ExternalInput")
    d_seln = nc.dram_tensor("selneg", [128, 128], BF16, kind="ExternalInput")
    d_id = nc.dram_tensor("ident", [128, 128], BF16, kind="ExternalInput")
    d_id32 = nc.dram_tensor("ident32", [128, 128], F32, kind="ExternalInput")
    d_out = nc.dram_tensor("out", [NPC, N], F32, kind="ExternalOutput")

    d_hsl = [nc.dram_tensor(f"hsl{l}", [NPC, C], BF16) for l in range(2)]
    d_hall = [nc.dram_tensor(f"hall{l}", [N, C], BF16, addr_space="Shared")
              for l in range(2)]
    d_hslT = nc.dram_tensor("hslT", [C, NPC], BF16)
    d_hallT = nc.dram_tensor("hallT", [NCORES * C, NPC], BF16,
                             addr_space="Shared")
    d_htab = [nc.dram_tensor(f"htab{l}", [N, 128], BF16) for l in range(2)]

    RG = [list(range(NCORES))]

    with tile.TileContext(nc) as tc:
        PF = TC
        with (
            tc.tile_pool(name="const", bufs=1) as cp,
            tc.tile_pool(name="hgp", bufs=2) as hgp,
            tc.tile_pool(name="msgp", bufs=6) as msgp,
            tc.tile_pool(name="wrp", bufs=3) as wrp,
            tc.tile_pool(name="pfp", bufs=1) as pfp,
            tc.tile_pool(name="tp", bufs=4) as tpp,
            tc.tile_pool(name="hcp", bufs=6) as hcp,
            tc.tile_pool(name="smf", bufs=6) as smf,
            tc.tile_pool(name="zp", bufs=2, space="PSUM") as zp,
            tc.tile_pool(name="aggp", bufs=1, space="PSUM") as aggp,
            tc.tile_pool(name="smp", bufs=1, space="PSUM") as smp,
        ):
            def cload(dram, shape, dtype, tag, eng=None):
                t = cp.tile(shape, dtype, tag=tag)
                (eng or nc.sync).dma_start(out=t[:], in_=dram.ap())
                return t

            # L1-critical constants first (SP queue), everything L1 doesn't
            # need goes on the idle Pool queue so L1 can start immediately.
            ea_sb = cload(d_eaT7, [7, Ep], BF16, "ea")
            xs_sb = cload(d_xsrc, [128, Tt], F32, "xs")
            w1_sb = cload(d_W1, [7, C], BF16, "w1")
            # S split into 4 column chunks so L1 can start on the first
            # quarter while the rest streams in.
            S_sb = cp.tile([128, Tt * 128], BF16, tag="S")
            qw = (Tt * 128) // 4
            for _q in range(2):
                nc.sync.dma_start(
                    out=S_sb[:, _q * qw:(_q + 1) * qw],
                    in_=d_S.ap()[:, _q * qw:(_q + 1) * qw])
            xT2_sb = cload(d_xT2, [2, NPC], F32, "xT2")
            invd_sb = cload(d_invd, [128, 2], F32, "invd")
            r1_sb = cload(d_r1, [2, C], F32, "r1")
            id32_sb = cload(d_id32, [128, 128], F32, "id32")
            for _q in range(2, 4):
                nc.sync.dma_start(
                    out=S_sb[:, _q * qw:(_q + 1) * qw],
                    in_=d_S.ap()[:, _q * qw:(_q + 1) * qw])
            ix_sb = cload(d_idx, [128, Tt * 8], I16, "ix", nc.gpsimd)
            w2_sb = cload(d_W2, [7, C * C], BF16, "w2", nc.gpsimd)
            w3_sb = cload(d_W3, [7, C * C], BF16, "w3", nc.gpsimd)
            r2_sb = cload(d_r2, [C + 1, C], F32, "r2", nc.gpsimd)
            r3_sb = cload(d_r3, [C + 1, C], F32, "r3", nc.gpsimd)
            sel_sb = cload(d_sel, [128, 252], BF16, "sel", nc.gpsimd)
            seln_sb = cload(d_seln, [128, 128], BF16, "seln", nc.gpsimd)
            id_sb = cload(d_id, [128, 128], BF16, "id", nc.gpsimd)

            hT1 = cp.tile([C + 1, NPC], F32, tag="hT1")
            hT2 = cp.tile([C + 1, NPC], F32, tag="hT2")
            hT3 = cp.tile([C, NPC], F32, tag="hT3")
            hT3b = cp.tile([C, NPC], BF16, tag="hT3b")
            Rloc = cp.tile([128, 2], F32, tag="Rloc")
            nc.vector.memset(hT1[C:C + 1, :], 1.0)
            nc.vector.memset(hT2[C:C + 1, :], 1.0)

            # ---------------- layer 1 (in_c = 1) ----------------
            for ch in range(2):
                agg = aggp.tile([128, C], F32, tag="aggw")
                for t in range(TC):
                    gt = ch * TC + t
                    z1 = zp.tile([128, C], F32, tag="z")
                    nc.tensor.matmul(z1[:], ea_sb[:, gt * 128:(gt + 1) * 128],
                                     w1_sb[:], start=True, stop=True)
                    msg = msgp.tile([128, C], BF16, tag="msg")
                    nc.vector.tensor_scalar(
                        msg[:], z1[:], 0.0, xs_sb[:, gt:gt + 1],
                        Op.max, Op.mult)
                    nc.tensor.matmul(agg[:], S_sb[:, gt * 128:(gt + 1) * 128],
                                     msg[:], start=(t == 0), stop=(t == TC - 1))
                rtp = smp.tile([128, C], F32, tag="root")
                nc.tensor.matmul(rtp[:], xT2_sb[:, ch * 128:(ch + 1) * 128],
                                 r1_sb[:], start=True, stop=True)
                sm = smf.tile([128, C], F32, tag="sm")
                nc.vector.tensor_scalar(sm[:], agg[:],
                                        invd_sb[:, ch:ch + 1], None, Op.mult)
                hf_c = hcp.tile([128, C], F32, tag="hf")
                nc.vector.tensor_tensor(hf_c[:], sm[:], rtp[:], Op.add)
                nc.vector.tensor_scalar(hf_c[:], hf_c[:], 0.0, None, Op.max)
                h_c = hcp.tile([128, C], BF16, tag="hc")
                nc.scalar.copy(h_c[:], hf_c[:])
                tp = smp.tile([32, 128], F32, tag="tp")
                nc.tensor.transpose(tp[:], hf_c[:], id32_sb[:])
                nc.scalar.copy(hT1[0:C, ch * 128:(ch + 1) * 128], tp[:])
                nc.sync.dma_start(out=d_hsl[0][ch * 128:(ch + 1) * 128, :],
                                  in_=h_c[:])

            # z = relu(ea@W) for the next layer's first PF tiles is
            # independent of the collective -- precompute it so PE/ACT/DVE
            # stay busy during the AllGather window.
            def z_prefetch(w_sb):
                tiles = []
                for t in range(PF):
                    z = zp.tile([128, C * C], F32, tag="z")
                    for q in range(2):
                        nc.tensor.matmul(
                            z[:, q * 512:(q + 1) * 512],
                            ea_sb[:, t * 128:(t + 1) * 128],
                            w_sb[:, q * 512:(q + 1) * 512],
                            start=True, stop=True)
                    wr = pfp.tile([128, C * C], BF16, tag=f"pf{t}")
                    if t % 2 == 1:
                        nc.vector.tensor_scalar(wr[:], z[:], 0.0, None,
                                                Op.max)
                    else:
                        nc.scalar.activation(
                            wr[:], z[:], mybir.ActivationFunctionType.Relu)
                    tiles.append(wr)
                return tiles

            pf_next = z_prefetch(w2_sb)
            nc.gpsimd.collective_compute(
                "AllGather", Op.bypass, replica_groups=RG,
                ins=[d_hsl[0].ap()], outs=[d_hall[0].ap()])
            htab_engs = [nc.sync, nc.scalar, nc.gpsimd, nc.scalar]
            for k in range(4):
                htab_engs[k].dma_start(out=d_htab[0][:, k * C:(k + 1) * C],
                                       in_=d_hall[0].ap())

            # ---------------- layers 2 and 3 ----------------
            for li, (w_sb, r_sb, hTprev, hTcur) in enumerate(
                    [(w2_sb, r2_sb, hT1, hT2), (w3_sb, r3_sb, hT2, hT3)]):
                tab = d_htab[li]
                # both chunks' gathers issued up front: ch=1's transfer hides
                # behind ch=0 compute.
                hgs = []
                for ch in range(2):
                    hg = hgp.tile([128, TC, 128], BF16, tag="hg")
                    # <=512 indices per dma_gather: one 4224-idx gather
                    # overflows the SWDGE descriptor ring (hang).
                    for g in range((TC * 128 + 511) // 512):
                        n_idx = min(512, TC * 128 - g * 512)
                        base = ch * TC * 8 + g * 32
                        nc.gpsimd.dma_gather(
                            out_ap=hg[:, g * 4:g * 4 + (n_idx + 127) // 128, :],
                            in_ap=tab.ap(),
                            idxs_ap=ix_sb[:, base:base + (n_idx + 15) // 16],
                            num_idxs=n_idx, num_idxs_reg=n_idx,
                            elem_size=128)
                    hgs.append(hg)
                for ch in range(2):
                    hg = hgs[ch]
                    aggw = aggp.tile([128, C * C], F32, tag="aggw")
                    for t in range(TC):
                        gt = ch * TC + t
                        if ch == 0 and t < PF:
                            wr = pf_next[t]
                        else:
                            z = zp.tile([128, C * C], F32, tag="z")
                            for q in range(2):
                                nc.tensor.matmul(
                                    z[:, q * 512:(q + 1) * 512],
                                    ea_sb[:, gt * 128:(gt + 1) * 128],
                                    w_sb[:, q * 512:(q + 1) * 512],
                                    start=True, stop=True)
                            wr = wrp.tile([128, C * C], BF16, tag="wr")
                            if t % 5 == 1:
                                nc.vector.tensor_scalar(
                                    wr[:], z[:], 0.0, None, Op.max)
                            else:
                                nc.scalar.activation(
                                    wr[:], z[:],
                                    mybir.ActivationFunctionType.Relu)
                        tt = tpp.tile([128, C * C], BF16, tag="t")
                        t3 = tt[:].rearrange("p (o i) -> p o i", i=C)
                        hgb = hg[:, t:t + 1, 0:C].broadcast_to([128, C, C])
                        wr3 = wr[:].rearrange("p (o i) -> p o i", i=C)
                        nc.vector.tensor_tensor(t3, wr3, hgb, Op.mult)
                        for q in range(2):
                            nc.tensor.matmul(
                                aggw[:, q * 512:(q + 1) * 512],
                                S_sb[:, gt * 128:(gt + 1) * 128],
                                tt[:, q * 512:(q + 1) * 512],
                                start=(t == 0), stop=(t == TC - 1))
                    rtp = smp.tile([128, C], F32, tag="root")
                    nc.tensor.matmul(rtp[:],
                                     hTprev[:, ch * 128:(ch + 1) * 128],
                                     r_sb[:], start=True, stop=True)
                    red = smf.tile([128, C], F32, tag="red")
                    nc.vector.tensor_reduce(
                        red[:], aggw[:].rearrange("p (o i) -> p o i", i=C),
                        mybir.AxisListType.X, Op.add)
                    sm = smf.tile([128, C], F32, tag="sm")
                    nc.vector.tensor_scalar(sm[:], red[:],
                                            invd_sb[:, ch:ch + 1], None,
                                            Op.mult)
                    hf_c = hcp.tile([128, C], F32, tag="hf")
                    nc.vector.tensor_tensor(hf_c[:], sm[:], rtp[:], Op.add)
                    nc.vector.tensor_scalar(hf_c[:], hf_c[:], 0.0, None,
                                            Op.max)
                    if li == 1:
                        nc.vector.tensor_reduce(
                            Rloc[:, ch:ch + 1], hf_c[:],
                            mybir.AxisListType.X, Op.add)
                    tp = smp.tile([32, 128], F32, tag="tp")
                    nc.tensor.transpose(tp[:], hf_c[:], id32_sb[:])
                    nc.scalar.copy(hTcur[0:C, ch * 128:(ch + 1) * 128], tp[:])
                    if li == 0:
                        h_c = hcp.tile([128, C], BF16, tag="hc")
                        nc.scalar.copy(h_c[:], hf_c[:])
                        nc.sync.dma_start(
                            out=d_hsl[1][ch * 128:(ch + 1) * 128, :],
                            in_=h_c[:])
                    else:
                        # transposed bf16 copy for the final AllGather
                        nc.scalar.copy(
                            hT3b[0:C, ch * 128:(ch + 1) * 128], tp[:])
                if li == 0:
                    pf_next = z_prefetch(w3_sb)
                    nc.gpsimd.collective_compute(
                        "AllGather", Op.bypass, replica_groups=RG,
                        ins=[d_hsl[1].ap()], outs=[d_hall[1].ap()])
                    for k in range(4):
                        htab_engs[k].dma_start(
                            out=d_htab[1][:, k * C:(k + 1) * C],
                            in_=d_hall[1].ap())

            # ---------------- CBT prep ----------------
            # htcols only needs LOCAL h3 -- build it before the AllGather so
            # the work overlaps the collective.  (Stays f32: tensor_scalar
            # scalar operands must be f32, and they don't gate perf modes.)
            htcols = cp.tile([128, NPC // 4], F32, tag="htcols")
            for s in range(4):
                nc.sync.dma_start(
                    out=htcols[s * 32:(s + 1) * 32, :],
                    in_=hT3[0:C, s::4])
            nc.sync.dma_start(out=d_hslT.ap(), in_=hT3b[:])
            nc.gpsimd.collective_compute(
                "AllGather", Op.bypass, replica_groups=RG,
                ins=[d_hslT.ap()], outs=[d_hallT.ap()])
            # gathered [8*32, 256] -> [32, 2048] node-major stripe, then
            # replicate to all 4 partition quadrants with parallel DMAs.
            hTrep = cp.tile([128, N], BF16, tag="hTrep")
            nc.sync.dma_start(
                out=hTrep[0:32, :].rearrange("k (r e) -> k r e", r=NCORES),
                in_=d_hallT.ap().rearrange("(r k) e -> k r e", r=NCORES))
            nc.sync.dma_start(out=hTrep[32:64, :], in_=hTrep[0:32, :])
            nc.scalar.dma_start(out=hTrep[64:96, :], in_=hTrep[0:32, :])
            nc.gpsimd.dma_start(out=hTrep[96:128, :], in_=hTrep[0:32, :])

        # ---------------- CBT ----------------
        with (
            tc.tile_pool(name="ap_", bufs=3) as apool,
            tc.tile_pool(name="ob", bufs=4) as ob,
            tc.tile_pool(name="cbtp", bufs=2, space="PSUM") as cbtp,
        ):
            for ib in range(2):
                cps = cbtp.tile([128, N], F32, tag="cps")
                for b in range(32):
                    col = ib * 32 + b
                    a_b = apool.tile([128, N], BF16, tag="a")
                    nc.vector.tensor_scalar(
                        a_b[:], hTrep[:], htcols[:, col:col + 1], 0.0,
                        Op.subtract, Op.max)
                    lo = 124 - 4 * b
                    for q in range(4):
                        nc.tensor.matmul(
                            cps[:, q * 512:(q + 1) * 512],
                            sel_sb[:, lo:lo + 128],
                            a_b[:, q * 512:(q + 1) * 512],
                            start=(b == 0), stop=False)
                for q in range(4):
                    nc.tensor.matmul(
                        cps[:, q * 512:(q + 1) * 512], seln_sb[:],
                        hTrep[:, q * 512:(q + 1) * 512],
                        start=False, stop=True)
                for q in range(4):
                    ob_t = ob.tile([128, 512], F32, tag="ob")
                    if q % 2 == 0:
                        nc.scalar.activation(
                            ob_t[:], cps[:, q * 512:(q + 1) * 512],
                            mybir.ActivationFunctionType.Identity,
                            bias=Rloc[:, ib:ib + 1], scale=2.0)
                    else:
                        nc.vector.tensor_scalar(
                            ob_t[:], cps[:, q * 512:(q + 1) * 512],
                            2.0, Rloc[:, ib:ib + 1], Op.mult, Op.add)
                    # spread the output stores over SP/ACT/Pool so the last
                    # block's drain isn't serialized on one queue
                    oeng = [nc.sync, nc.scalar, nc.gpsimd, nc.scalar][q]
                    oeng.dma_start(
                        out=d_out[ib * 128:(ib + 1) * 128,
                                  q * 512:(q + 1) * 512],
                        in_=ob_t[:])

    nc.compile()
    return nc


# --------------------------------------------------------------------------
# entry point
# --------------------------------------------------------------------------

def kernel(**inputs):
    TC, in_maps = _prep(**inputs)
    nc = _PROGRAM_CACHE.get(TC)
    if nc is None:
        nc = build_program(TC)
        _PROGRAM_CACHE[TC] = nc
    res = run_bass_kernel_spmd(nc, in_maps, list(range(NCORES)))
    out = np.concatenate([res.results[c]["out"] for c in range(NCORES)], 0)
    return out.astype(np.float32)


if __name__ == "__main__":
    rng = np.random.default_rng(0)
    pass

